# revision 42
# baseline (speedup 1.0000x reference)
"""Causal self-attention (B=4, T=2048, C=1024, H=16) on 8 trn2 NeuronCores.

Sharding: 4 batches x 2 head-groups (8 heads each). Each core computes the
row-parallel partial of the output projection for its (batch, head-group);
the host sums the two (bf16) partials per batch in f32 and folds all
biases in exactly.

Precision: the QK^T projection runs in fp8 e4m3 with perf_mode=DoubleRow
(2 MACs/cell, halving its PE stream time; weights are scaled x64 into the
e4m3 normal range and the x64^2 undone inside the exp scale). Everything
else runs in bf16 (fp8 on the V/P path would blow the 2e-2 gate). The
host supplies x pre-transposed in both formats, so there are no PE
transposes; all weights stay cached in SBUF across t-groups.

Schedule notes (these carried ~25% of the win):
- input DMAs ride the Activation HWDGE queue, output DMAs the SP queue:
  one queue is FIFO with head-of-line blocking, so an output DMA waiting
  on compute would stall the next t-group's x^T prefetch.
- the output projection of t-group g is deferred and interleaved between
  the attention pairs of t-group g+1: exp on ScalarE is the attention
  pipeline's limiter, and the PE queue is strict FIFO, so the proj
  matmuls are the only work that can fill those bubbles.
- per-t-group K^T/V caches double-buffer across benchmark repeats so the
  repeat loop pipelines (no WAR serialization at the boundary).

Per-core device pipeline per 512-query t-group:
  DMA x^T slice -> fp8 DoubleRow QK^T projection (Q^T/K^T in [feature, t]
  layout, two heads packed per 128-partition block) -> bf16 V projection
  (ones column appended for the softmax denominator) -> causal
  S^T = K^T.T @ Q^T with two heads per matmul via tile_position row strips
  (hd=64), exact causal widths -> one fused exp per S tile on ScalarE
  (both head strips in one instruction; no max-subtraction needed since
  scores are O(1)) -> AV matmul whose ones row yields the denominator for
  free -> normalize via reciprocal + gpsimd partition broadcast ->
  interleaved row-parallel output projection -> bf16 out DMA.
"""

from contextlib import ExitStack

import ml_dtypes
import numpy as np

import concourse.bacc as bacc
import concourse.mybir as mybir
import concourse.tile as tile
from concourse.bass_utils import run_bass_kernel_spmd
from concourse.masks import make_upper_triangular

f32 = mybir.dt.float32
bf16 = mybir.dt.bfloat16
f8 = mybir.dt.float8e4
DR = mybir.MatmulPerfMode.DoubleRow
Exp = mybir.ActivationFunctionType.Exp

BF16 = ml_dtypes.bfloat16
F8 = ml_dtypes.float8_e4m3  # TRN e4m3: max normal +-240

B, T, C = 4, 2048, 1024
H, HD = 16, 64
G = 2                      # head groups across cores
HPG = H // G               # 8 heads per group
NPAIR = HPG // 2           # 4 head pairs per group
NCORES = B * G             # 8
TGS = 512                  # t-group size
NTG = T // TGS             # 4 t-groups
WS = 64.0                  # fp8 weight upscale (w std 0.02 is subnormal in e4m3)
SCALE = 1.0 / np.sqrt(HD)  # 0.125
SCALE8 = SCALE / (WS * WS)  # Q,K both carry the x64 weight scale


def build_kernel(ctx, tc, repeat=1):
    nc = tc.nc
    xt_d = nc.dram_tensor("xt", [C, T], bf16, kind="ExternalInput")
    # fp8 copies for the DoubleRow QK projection, pre-arranged on the host
    # into the paired-contraction layout [p, ko2, s, t] (c = ko2*256+s*128+p)
    xt8_d = nc.dram_tensor("xt8", [128, 4, 2, T], f8, kind="ExternalInput")
    wqk8_d = nc.dram_tensor("wqk8", [128, 4, 2, 1024], f8, kind="ExternalInput")
    wv_d = nc.dram_tensor("wv", [C, 512], bf16, kind="ExternalInput")
    wp_d = nc.dram_tensor("wp", [512, C], bf16, kind="ExternalInput")
    bqk_d = nc.dram_tensor("bqk", [128, 8], f32, kind="ExternalInput")
    out_d = nc.dram_tensor("out", [T, C], bf16, kind="ExternalOutput")

    const = ctx.enter_context(tc.tile_pool(name="const", bufs=1))
    cache = ctx.enter_context(tc.tile_pool(name="cache", bufs=1))
    xTp = ctx.enter_context(tc.tile_pool(name="xT", bufs=2))
    xT8p = ctx.enter_context(tc.tile_pool(name="xT8", bufs=2))
    qtp = ctx.enter_context(tc.tile_pool(name="qt", bufs=8))
    ytp = ctx.enter_context(tc.tile_pool(name="yt", bufs=8))
    ptp = ctx.enter_context(tc.tile_pool(name="pt", bufs=4))
    rbp = ctx.enter_context(tc.tile_pool(name="rb", bufs=2))

    poutp = ctx.enter_context(tc.tile_pool(name="pout", bufs=2))
    pps = ctx.enter_context(tc.tile_pool(name="pps", bufs=2, space="PSUM"))
    sps = ctx.enter_context(tc.tile_pool(name="sps", bufs=2, space="PSUM"))
    avps = ctx.enter_context(tc.tile_pool(name="avps", bufs=1, space="PSUM"))

    # persistent caches (bf16, loaded once per execution). KT/Vaug alternate
    # between two buffers per repeat so benchmark repeats pipeline instead of
    # serializing on the WAR hazard at the repeat boundary (repeat=1 unaffected).
    nrep_bufs = min(repeat, 2)
    KTs = [
        cache.tile([128, NPAIR, T], bf16, name=f"KT{r}") for r in range(nrep_bufs)
    ]
    Vaugs = [
        cache.tile([128, HPG, 16, 65], bf16, name=f"Vaug{r}")
        for r in range(nrep_bufs)
    ]
    Wqk8 = cache.tile([128, 4, 2, 1024], f8)
    Wv = cache.tile([128, 8, 512], bf16)
    Wp = cache.tile([128, NPAIR, 1024], bf16)

    # inputs go through the Activation HWDGE queue, outputs through SP:
    # a single queue is FIFO with head-of-line blocking, so an output DMA
    # waiting on compute would stall the next t-group's x^T prefetch kick.
    # First x^T slices ahead of everything: they gate the first matmuls.
    xt_r = xt_d.rearrange("(cb p) t -> p cb t", p=128)
    bqk_sb = const.tile([128, 8], f32)
    nc.scalar.dma_start(bqk_sb[:], bqk_d[:])
    xT80 = xT8p.tile([128, 4, 2, TGS], f8)
    nc.scalar.dma_start(xT80[:], xt8_d[:, :, :, 0:TGS])
    nc.scalar.dma_start(Wqk8[:, :, :, 0:512], wqk8_d[:, :, :, 0:512])    # Q
    nc.scalar.dma_start(Wqk8[:, :, :, 512:1024], wqk8_d[:, :, :, 512:1024])  # K
    xT0 = xTp.tile([128, 8, TGS], bf16)
    nc.scalar.dma_start(xT0[:], xt_r[:, :, 0:TGS])
    nc.scalar.dma_start(Wv[:], wv_d.rearrange("(ko p) n -> p ko n", p=128))
    nc.scalar.dma_start(Wp[:], wp_d.rearrange("(ko p) n -> p ko n", p=128))

    # constants: causal mask replicated for the two packed head strips
    tri2 = const.tile([128, 2, 128], bf16)
    for s in range(2):
        make_upper_triangular(nc, tri2[:, s, :], val=1.0, diag=True)
    onesj = const.tile([128, 16], bf16)
    nc.any.memset(onesj[:], 1.0)
    for Vaug in Vaugs:
        for h in range(HPG):
            nc.vector.tensor_copy(Vaug[:, h, :, 64], onesj[:, :])

    def proj_group(g, tl, cg, ytg, drain=False):
        # one output-projection tile of t-group g: 4 accumulating matmuls,
        # bf16 copy, DMA out (on the SP queue). The copy runs on DVE: an
        # ACT-queued copy would block later exps (FIFO queue). Only drain
        # groups (no exps after them) alternate onto ACT to halve the tail.
        tb = 4 * g + tl
        ps2 = pps.tile([128, 512], f32, tag="pps")
        for pair in range(NPAIR):
            nc.tensor.matmul(
                ps2[:], ytg[pair][:, tl * 128:(tl + 1) * 128],
                Wp[:, pair, cg * 512:(cg + 1) * 512],
                start=(pair == 0), stop=(pair == NPAIR - 1),
            )
        po = poutp.tile([128, 512], bf16, tag="po")
        if drain and cg == 0:
            nc.scalar.copy(po[:], ps2[:])
        else:
            nc.vector.tensor_copy(po[:], ps2[:])
        nc.sync.dma_start(
            out_d[tb * 128:(tb + 1) * 128, cg * 512:(cg + 1) * 512], po[:]
        )

    gs = [g for _ in range(repeat) for g in range(NTG)]
    xT_next, xT8_next = xT0, xT80
    prev = None  # (g, ytg) of the previous iteration, projection still owed
    for it, g in enumerate(gs):
        gq = slice(g * TGS, (g + 1) * TGS)
        KT = KTs[(it // NTG) % nrep_bufs]
        Vaug = Vaugs[(it // NTG) % nrep_bufs]
        xTg, xT8g = xT_next, xT8_next

        # ---- QK^T projection (fp8 DoubleRow, 2 MACs/cell): out holds
        # WS^2-scaled Q/K; the scale is folded into SCALE8 at the exp ----
        qts = []
        for cb in range(8):
            ps_ = pps.tile([128, TGS], f32, tag="pps")
            for ko2 in range(4):
                nc.tensor.matmul(
                    ps_[:], Wqk8[:, ko2, :, cb * 128:(cb + 1) * 128],
                    xT8g[:, ko2, :, :],
                    start=(ko2 == 0), stop=(ko2 == 3), perf_mode=DR,
                )
            if cb < 4:   # Q pair block (attention scale is applied inside exp)
                qt = qtp.tile([128, TGS], bf16)
                nc.vector.tensor_scalar_add(qt[:], ps_[:], bqk_sb[:, cb:cb + 1])
                qts.append(qt)
            else:        # K pair block
                nc.vector.tensor_scalar_add(
                    KT[:, cb - 4, gq], ps_[:], bqk_sb[:, cb:cb + 1]
                )

        # prefetch next t-group's x^T now: the kicks must precede this
        # t-group's exp instructions in the ACT queue's program order
        if it + 1 < len(gs):
            gn = gs[it + 1]
            gnq = slice(gn * TGS, (gn + 1) * TGS)
            xT8_next = xT8p.tile([128, 4, 2, TGS], f8)
            nc.scalar.dma_start(xT8_next[:], xt8_d[:, :, :, gnq])
            xT_next = xTp.tile([128, 8, TGS], bf16)
            nc.scalar.dma_start(xT_next[:], xt_r[:, :, gnq])

        # ---- V projection: out [t, v-col] ----
        for tl in range(4):
            j = 4 * g + tl
            ps_ = pps.tile([128, TGS], f32, tag="pps")
            for ko in range(8):
                nc.tensor.matmul(
                    ps_[:], xTg[:, ko, tl * 128:(tl + 1) * 128], Wv[:, ko, :],
                    start=(ko == 0), stop=(ko == 7),
                )
            nc.vector.tensor_copy(
                Vaug[:, :, j, 0:64], ps_[:].rearrange("p (h d) -> p h d", h=8)
            )

        # ---- attention for q-group g, with the previous t-group's output
        # projection interleaved between pairs: those matmuls depend only on
        # ytg(g-1), so they fill the PE bubbles left while ScalarE works
        # through this t-group's exp stream ----
        ytg = []
        for pair in range(NPAIR):
            if prev is not None:
                pg, pytg = prev
                proj_group(pg, pair, 0, pytg)
                proj_group(pg, pair, 1, pytg)
            qt = qts[pair]
            av0 = avps.tile([65, TGS], f32, tag="av0")
            av1 = avps.tile([65, TGS], f32, tag="av1")
            nj = 4 * g + 4
            for j in range(nj):
                c0 = (j - 4 * g) * 128 if j >= 4 * g else 0
                jsl = slice(j * 128, (j + 1) * 128)
                sp = sps.tile([128, 2, TGS], f32, tag="sp")
                nc.tensor.matmul(
                    sp[:, 0, c0:TGS], KT[0:64, pair, jsl], qt[0:64, c0:TGS],
                    start=True, stop=True, tile_position=(0, 0),
                )
                nc.tensor.matmul(
                    sp[:, 1, c0:TGS], KT[64:128, pair, jsl], qt[64:128, c0:TGS],
                    start=True, stop=True, tile_position=(64, 0),
                )
                pt = ptp.tile([128, 2, TGS], bf16)
                nc.scalar.activation(
                    pt[:, :, c0:TGS], sp[:, :, c0:TGS], Exp, scale=SCALE8
                )
                if j >= 4 * g:  # diagonal block: causal mask (keep tk <= tq)
                    nc.gpsimd.tensor_mul(
                        pt[:, :, c0:c0 + 128], pt[:, :, c0:c0 + 128], tri2[:]
                    )
                nc.tensor.matmul(
                    av0[:, c0:TGS], Vaug[:, 2 * pair, j, :], pt[:, 0, c0:TGS],
                    start=(j == 0), stop=(j == nj - 1),
                )
                nc.tensor.matmul(
                    av1[:, c0:TGS], Vaug[:, 2 * pair + 1, j, :], pt[:, 1, c0:TGS],
                    start=(j == 0), stop=(j == nj - 1),
                )
            # normalize: row 64 of av psums holds the softmax denominator.
            # Per-strip recip -> broadcast -> mul so the three engines
            # pipeline instead of serializing the whole chain.
            rb_ = rbp.tile([128, 2, TGS], f32)
            yt_p = ytp.tile([128, TGS], bf16)
            nc.vector.reciprocal(rb_[0:1, 0, :], av0[64:65, :])
            nc.gpsimd.partition_broadcast(rb_[:, 0, :], rb_[0:1, 0, :])
            nc.vector.reciprocal(rb_[0:1, 1, :], av1[64:65, :])
            nc.vector.tensor_mul(yt_p[0:64, :], av0[0:64, :], rb_[0:64, 0, :])
            nc.gpsimd.partition_broadcast(rb_[:, 1, :], rb_[0:1, 1, :])
            nc.vector.tensor_mul(yt_p[64:128, :], av1[0:64, :], rb_[64:128, 1, :])
            ytg.append(yt_p)

        prev = (g, ytg)

    # drain the last t-group's projection
    pg, pytg = prev
    for tl in range(4):
        for cg in range(2):
            proj_group(pg, tl, cg, pytg, drain=True)


_NC = {}


def get_nc(repeat=1):
    if repeat not in _NC:
        nc = bacc.Bacc("TRN2", target_bir_lowering=False, debug=False)
        with tile.TileContext(nc) as tc, ExitStack() as ctx:
            build_kernel(ctx, tc, repeat=repeat)
        nc.compile()
        _NC[repeat] = nc
    return _NC[repeat]


def make_in_maps(x, w_attn, b_attn, w_proj):
    x = np.asarray(x, np.float32)
    w_attn = np.asarray(w_attn, np.float32)
    b_attn = np.asarray(b_attn, np.float32)
    w_proj = np.asarray(w_proj, np.float32)
    def to_dr(a):
        # [C, n] -> paired-contraction fp8 layout [p, ko2, s, n], c=ko2*256+s*128+p
        n = a.shape[1]
        q = np.clip(a, -240, 240).astype(F8)
        return np.ascontiguousarray(q.reshape(4, 2, 128, n).transpose(2, 0, 1, 3))

    in_maps = []
    xts = [np.ascontiguousarray(x[b].T).astype(BF16) for b in range(B)]
    xt8s = [to_dr(np.asarray(x[b].T, np.float32)) for b in range(B)]
    for core in range(NCORES):
        b, g = divmod(core, G)
        wqk = np.concatenate(
            [w_attn[:, g * 512:(g + 1) * 512], w_attn[:, 1024 + g * 512:1024 + (g + 1) * 512]],
            axis=1,
        )
        wqk8 = to_dr(wqk * WS)
        wv = w_attn[:, 2048 + g * 512:2048 + (g + 1) * 512].astype(BF16)
        wp = w_proj[g * 512:(g + 1) * 512, :].astype(BF16)
        cols = []
        for cb in range(4):
            cols.append(b_attn[g * 512 + cb * 128: g * 512 + (cb + 1) * 128])
        for cb in range(4):
            cols.append(b_attn[1024 + g * 512 + cb * 128: 1024 + g * 512 + (cb + 1) * 128])
        bqk = (np.stack(cols, axis=1) * WS).astype(np.float32)
        in_maps.append(
            {"xt": xts[b], "xt8": xt8s[b], "wqk8": wqk8, "wv": wv, "wp": wp, "bqk": bqk}
        )
    return in_maps


def kernel(x, w_attn, b_attn, w_proj, b_proj):
    x = np.asarray(x, np.float32)
    w_attn = np.asarray(w_attn, np.float32)
    b_attn = np.asarray(b_attn, np.float32)
    w_proj = np.asarray(w_proj, np.float32)
    b_proj = np.asarray(b_proj, np.float32)

    nc = get_nc()
    in_maps = make_in_maps(x, w_attn, b_attn, w_proj)

    res = run_bass_kernel_spmd(nc, in_maps, list(range(NCORES))).results

    # v-bias contributes b_v @ w_proj to every output row; add with b_proj.
    bias_total = (b_proj + b_attn[2048:] @ w_proj).astype(np.float32)
    out = np.empty((B, T, C), np.float32)
    for b in range(B):
        out[b] = (
            res[G * b]["out"].astype(np.float32)
            + res[G * b + 1]["out"].astype(np.float32)
            + bias_total
        )
    return out


# revision 57
# speedup vs baseline: 1.3750x; 1.3750x over previous
"""Causal self-attention (B=4, T=2048, C=1024, H=16) on 8 trn2 NeuronCores.

Sharding: 4 batches x 2 head-groups (8 heads each). Each core computes the
row-parallel partial of the output projection for its (batch, head-group);
the host sums the two (bf16) partials per batch in f32 and folds all
biases in exactly.

Precision: the QK^T projection runs in fp8 e4m3 with perf_mode=DoubleRow
(2 MACs/cell, halving its PE stream time; weights are scaled x64 into the
e4m3 normal range and the x64^2 undone inside the exp scale). Everything
else runs in bf16 (fp8 on the V/P path would blow the 2e-2 gate). The
host supplies x pre-transposed in both formats, so there are no PE
transposes; all weights stay cached in SBUF across t-groups.

Schedule notes (these carried ~25% of the win):
- input DMAs ride the Activation HWDGE queue, output DMAs the SP queue:
  one queue is FIFO with head-of-line blocking, so an output DMA waiting
  on compute would stall the next t-group's x^T prefetch.
- the output projection of t-group g is deferred and interleaved between
  the attention pairs of t-group g+1: exp on ScalarE is the attention
  pipeline's limiter, and the PE queue is strict FIFO, so the proj
  matmuls are the only work that can fill those bubbles.
- per-t-group K^T/V caches double-buffer across benchmark repeats so the
  repeat loop pipelines (no WAR serialization at the boundary).

Per-core device pipeline per 512-query t-group:
  DMA x^T slice -> fp8 DoubleRow QK^T projection (Q^T/K^T in [feature, t]
  layout, two heads packed per 128-partition block) -> bf16 V projection
  (ones column appended for the softmax denominator) -> causal
  S^T = K^T.T @ Q^T with two heads per matmul via tile_position row strips
  (hd=64), exact causal widths -> one fused exp per S tile on ScalarE
  (both head strips in one instruction; no max-subtraction needed since
  scores are O(1)) -> AV matmul whose ones row yields the denominator for
  free -> normalize via reciprocal + gpsimd partition broadcast ->
  interleaved row-parallel output projection -> bf16 out DMA.
"""

from contextlib import ExitStack

import ml_dtypes
import numpy as np

import concourse.bacc as bacc
import concourse.mybir as mybir
import concourse.tile as tile
from concourse.bass_utils import run_bass_kernel_spmd
from concourse.masks import make_upper_triangular

f32 = mybir.dt.float32
bf16 = mybir.dt.bfloat16
f8 = mybir.dt.float8e4
DR = mybir.MatmulPerfMode.DoubleRow
Exp = mybir.ActivationFunctionType.Exp

BF16 = ml_dtypes.bfloat16
F8 = ml_dtypes.float8_e4m3  # TRN e4m3: max normal +-240

B, T, C = 4, 2048, 1024
H, HD = 16, 64
G = 2                      # head groups across cores
HPG = H // G               # 8 heads per group
NPAIR = HPG // 2           # 4 head pairs per group
NCORES = B * G             # 8
TGS = 512                  # t-group size
NTG = T // TGS             # 4 t-groups
WS = 64.0                  # fp8 weight upscale (w std 0.02 is subnormal in e4m3)
SCALE = 1.0 / np.sqrt(HD)  # 0.125
SCALE8 = SCALE / (WS * WS)  # Q,K both carry the x64 weight scale


def build_kernel(ctx, tc, repeat=1):
    nc = tc.nc
    xt_d = nc.dram_tensor("xt", [C, T], bf16, kind="ExternalInput")
    # fp8 copies for the DoubleRow QK projection, pre-arranged on the host
    # into the paired-contraction layout [p, ko2, s, t] (c = ko2*256+s*128+p)
    xt8_d = nc.dram_tensor("xt8", [128, 4, 2, T], f8, kind="ExternalInput")
    wqk8_d = nc.dram_tensor("wqk8", [128, 4, 2, 1024], f8, kind="ExternalInput")
    wv_d = nc.dram_tensor("wv", [C, 512], bf16, kind="ExternalInput")
    wp_d = nc.dram_tensor("wp", [512, C], bf16, kind="ExternalInput")
    bqk_d = nc.dram_tensor("bqk", [128, 8], f32, kind="ExternalInput")
    out_d = nc.dram_tensor("out", [T, C], bf16, kind="ExternalOutput")

    const = ctx.enter_context(tc.tile_pool(name="const", bufs=1))
    cache = ctx.enter_context(tc.tile_pool(name="cache", bufs=1))
    xTp = ctx.enter_context(tc.tile_pool(name="xT", bufs=2))
    xT8p = ctx.enter_context(tc.tile_pool(name="xT8", bufs=2))
    qtp = ctx.enter_context(tc.tile_pool(name="qt", bufs=8))
    ytp = ctx.enter_context(tc.tile_pool(name="yt", bufs=8))
    ptp = ctx.enter_context(tc.tile_pool(name="pt", bufs=4))
    rbp = ctx.enter_context(tc.tile_pool(name="rb", bufs=2))

    poutp = ctx.enter_context(tc.tile_pool(name="pout", bufs=2))
    partp = ctx.enter_context(tc.tile_pool(name="part", bufs=8))
    pps = ctx.enter_context(tc.tile_pool(name="pps", bufs=2, space="PSUM"))
    sps = ctx.enter_context(tc.tile_pool(name="sps", bufs=2, space="PSUM"))
    avps = ctx.enter_context(tc.tile_pool(name="avps", bufs=1, space="PSUM"))

    # persistent caches (bf16, loaded once per execution). KT/Vaug alternate
    # between two buffers per repeat so benchmark repeats pipeline instead of
    # serializing on the WAR hazard at the repeat boundary (repeat=1 unaffected).
    nrep_bufs = min(repeat, 2)
    KTs = [
        cache.tile([128, NPAIR, T], bf16, name=f"KT{r}") for r in range(nrep_bufs)
    ]
    Vaugs = [
        cache.tile([128, HPG, 16, 65], bf16, name=f"Vaug{r}")
        for r in range(nrep_bufs)
    ]
    Wqk8 = cache.tile([128, 4, 2, 1024], f8)
    Wv = cache.tile([128, 8, 512], bf16)
    Wp = cache.tile([128, NPAIR, 1024], bf16)

    # inputs go through the Activation HWDGE queue, outputs through SP:
    # a single queue is FIFO with head-of-line blocking, so an output DMA
    # waiting on compute would stall the next t-group's x^T prefetch kick.
    # First x^T slices ahead of everything: they gate the first matmuls.
    xt_r = xt_d.rearrange("(cb p) t -> p cb t", p=128)
    bqk_sb = const.tile([128, 8], f32)
    nc.scalar.dma_start(bqk_sb[:], bqk_d[:])
    xT80 = xT8p.tile([128, 4, 2, TGS], f8)
    nc.scalar.dma_start(xT80[:], xt8_d[:, :, :, 0:TGS])
    nc.scalar.dma_start(Wqk8[:, :, :, 0:512], wqk8_d[:, :, :, 0:512])    # Q
    nc.scalar.dma_start(Wqk8[:, :, :, 512:1024], wqk8_d[:, :, :, 512:1024])  # K
    xT0 = xTp.tile([128, 8, TGS], bf16)
    nc.scalar.dma_start(xT0[:], xt_r[:, :, 0:TGS])
    nc.scalar.dma_start(Wv[:], wv_d.rearrange("(ko p) n -> p ko n", p=128))
    nc.scalar.dma_start(Wp[:], wp_d.rearrange("(ko p) n -> p ko n", p=128))

    # constants: causal mask replicated for the two packed head strips
    tri2 = const.tile([128, 2, 128], bf16)
    for s in range(2):
        make_upper_triangular(nc, tri2[:, s, :], val=1.0, diag=True)
    onesj = const.tile([128, 16], bf16)
    nc.any.memset(onesj[:], 1.0)
    for Vaug in Vaugs:
        for h in range(HPG):
            nc.vector.tensor_copy(Vaug[:, h, :, 64], onesj[:, :])

    def proj_group(g, tl, cg, ytg, drain=False):
        # one output-projection tile of t-group g: 4 accumulating matmuls,
        # bf16 copy, DMA out (on the SP queue). The copy runs on DVE: an
        # ACT-queued copy would block later exps (FIFO queue). Only drain
        # groups (no exps after them) alternate onto ACT to halve the tail.
        tb = 4 * g + tl
        ps2 = pps.tile([128, 512], f32, tag="pps")
        for pair in range(NPAIR):
            nc.tensor.matmul(
                ps2[:], ytg[pair][:, tl * 128:(tl + 1) * 128],
                Wp[:, pair, cg * 512:(cg + 1) * 512],
                start=(pair == 0), stop=(pair == NPAIR - 1),
            )
        po = poutp.tile([128, 512], bf16, tag="po")
        if drain and cg == 0:
            nc.scalar.copy(po[:], ps2[:])
        else:
            nc.vector.tensor_copy(po[:], ps2[:])
        nc.sync.dma_start(
            out_d[tb * 128:(tb + 1) * 128, cg * 512:(cg + 1) * 512], po[:]
        )

    def v_group(gn, tl, xTn, Vaug_n):
        # one V-projection tile of t-group gn: 8 accumulating matmuls + copy
        j = 4 * gn + tl
        ps_ = pps.tile([128, TGS], f32, tag="pps")
        for ko in range(8):
            nc.tensor.matmul(
                ps_[:], xTn[:, ko, tl * 128:(tl + 1) * 128], Wv[:, ko, :],
                start=(ko == 0), stop=(ko == 7),
            )
        nc.vector.tensor_copy(
            Vaug_n[:, :, j, 0:64], ps_[:].rearrange("p (h d) -> p h d", h=8)
        )

    gs = [g for _ in range(repeat) for g in range(NTG)]
    xT_next, xT8_next = xT0, xT80
    prev = None  # (g, ytg) of the previous iteration, projection still owed
    for it, g in enumerate(gs):
        gq = slice(g * TGS, (g + 1) * TGS)
        KT = KTs[(it // NTG) % nrep_bufs]
        Vaug = Vaugs[(it // NTG) % nrep_bufs]
        xTg, xT8g = xT_next, xT8_next

        # prefetch next t-group's x^T immediately: the kicks ride the input
        # (ACT) queue, and the tiles must land before this t-group's
        # attention starts consuming the interleaved V matmuls below
        if it + 1 < len(gs):
            gn = gs[it + 1]
            gnq = slice(gn * TGS, (gn + 1) * TGS)
            xT8_next = xT8p.tile([128, 4, 2, TGS], f8)
            nc.scalar.dma_start(xT8_next[:], xt8_d[:, :, :, gnq])
            xT_next = xTp.tile([128, 8, TGS], bf16)
            nc.scalar.dma_start(xT_next[:], xt_r[:, :, gnq])

        # ---- QK^T projection (fp8 DoubleRow, 2 MACs/cell): out holds
        # WS^2-scaled Q/K; the scale is folded into SCALE8 at the exp ----
        qts = []
        for cb in range(8):
            ps_ = pps.tile([128, TGS], f32, tag="pps")
            for ko2 in range(4):
                nc.tensor.matmul(
                    ps_[:], Wqk8[:, ko2, :, cb * 128:(cb + 1) * 128],
                    xT8g[:, ko2, :, :],
                    start=(ko2 == 0), stop=(ko2 == 3), perf_mode=DR,
                )
            if cb < 4:   # Q pair block (attention scale is applied inside exp)
                qt = qtp.tile([128, TGS], bf16)
                nc.vector.tensor_scalar_add(qt[:], ps_[:], bqk_sb[:, cb:cb + 1])
                qts.append(qt)
            else:        # K pair block
                nc.vector.tensor_scalar_add(
                    KT[:, cb - 4, gq], ps_[:], bqk_sb[:, cb:cb + 1]
                )

        # ---- V projection: out [t, v-col] ----
        for tl in range(4):
            v_group(g, tl, xTg, Vaug)

        # ---- attention for q-group g, with the previous t-group's output
        # projection interleaved between pairs: those matmuls depend only on
        # ytg(g-1), so they fill the PE bubbles left while ScalarE works
        # through this t-group's exp stream ----
        ytg = []
        for pair in range(NPAIR):
            if prev is not None:
                pg, pytg = prev
                proj_group(pg, pair, 0, pytg)
                proj_group(pg, pair, 1, pytg)
            qt = qts[pair]
            av0 = avps.tile([65, TGS], f32, tag="av0")
            av1 = avps.tile([65, TGS], f32, tag="av1")
            nj = 4 * g + 4
            for j in range(nj):
                c0 = (j - 4 * g) * 128 if j >= 4 * g else 0
                jsl = slice(j * 128, (j + 1) * 128)
                sp = sps.tile([128, 2, TGS], f32, tag="sp")
                nc.tensor.matmul(
                    sp[:, 0, c0:TGS], KT[0:64, pair, jsl], qt[0:64, c0:TGS],
                    start=True, stop=True, tile_position=(0, 0),
                )
                nc.tensor.matmul(
                    sp[:, 1, c0:TGS], KT[64:128, pair, jsl], qt[64:128, c0:TGS],
                    start=True, stop=True, tile_position=(64, 0),
                )
                pt = ptp.tile([128, 2, TGS], bf16)
                nc.scalar.activation(
                    pt[:, :, c0:TGS], sp[:, :, c0:TGS], Exp, scale=SCALE8
                )
                if j >= 4 * g:  # diagonal block: causal mask (keep tk <= tq)
                    nc.gpsimd.tensor_mul(
                        pt[:, :, c0:c0 + 128], pt[:, :, c0:c0 + 128], tri2[:]
                    )
                nc.tensor.matmul(
                    av0[:, c0:TGS], Vaug[:, 2 * pair, j, :], pt[:, 0, c0:TGS],
                    start=(j == 0), stop=(j == nj - 1),
                )
                nc.tensor.matmul(
                    av1[:, c0:TGS], Vaug[:, 2 * pair + 1, j, :], pt[:, 1, c0:TGS],
                    start=(j == 0), stop=(j == nj - 1),
                )
            # normalize: row 64 of av psums holds the softmax denominator.
            # Per-strip recip -> broadcast -> mul so the three engines
            # pipeline instead of serializing the whole chain.
            rb_ = rbp.tile([128, 2, TGS], f32)
            yt_p = ytp.tile([128, TGS], bf16)
            nc.vector.reciprocal(rb_[0:1, 0, :], av0[64:65, :])
            nc.gpsimd.partition_broadcast(rb_[:, 0, :], rb_[0:1, 0, :])
            nc.vector.reciprocal(rb_[0:1, 1, :], av1[64:65, :])
            nc.vector.tensor_mul(yt_p[0:64, :], av0[0:64, :], rb_[0:64, 0, :])
            nc.gpsimd.partition_broadcast(rb_[:, 1, :], rb_[0:1, 1, :])
            nc.vector.tensor_mul(yt_p[64:128, :], av1[0:64, :], rb_[64:128, 1, :])
            ytg.append(yt_p)

        prev = (g, ytg)

    # drain the last t-group's projection. Split so only pair 3's matmul and
    # a psum+partial add sit behind the final softmax normalize: the pair0-2
    # partials copy out of PSUM immediately (no pair-3 wait), letting the
    # 2-buffer psum rotation pre-run all eight groups during the attention
    # tail instead of serializing the whole drain behind it.
    pg, pytg = prev
    partials = []
    for k in range(8):
        tl, cg = divmod(k, 2)
        ps2 = pps.tile([128, 512], f32, tag="pps")
        for pair in range(3):
            nc.tensor.matmul(
                ps2[:], pytg[pair][:, tl * 128:(tl + 1) * 128],
                Wp[:, pair, cg * 512:(cg + 1) * 512],
                start=(pair == 0), stop=(pair == 2),
            )
        part = partp.tile([128, 512], bf16, tag="part")
        nc.vector.tensor_copy(part[:], ps2[:])
        partials.append(part)
    for k in range(8):
        tl, cg = divmod(k, 2)
        tb = 4 * pg + tl
        ps3 = pps.tile([128, 512], f32, tag="pps")
        nc.tensor.matmul(
            ps3[:], pytg[3][:, tl * 128:(tl + 1) * 128],
            Wp[:, 3, cg * 512:(cg + 1) * 512],
            start=True, stop=True,
        )
        po = poutp.tile([128, 512], bf16, tag="po")
        nc.vector.tensor_add(po[:], ps3[:], partials[k][:])
        nc.sync.dma_start(
            out_d[tb * 128:(tb + 1) * 128, cg * 512:(cg + 1) * 512], po[:]
        )


_NC = {}


def get_nc(repeat=1):
    if repeat not in _NC:
        nc = bacc.Bacc("TRN2", target_bir_lowering=False, debug=False)
        with tile.TileContext(nc) as tc, ExitStack() as ctx:
            build_kernel(ctx, tc, repeat=repeat)
        nc.compile()
        _NC[repeat] = nc
    return _NC[repeat]


def make_in_maps(x, w_attn, b_attn, w_proj):
    x = np.asarray(x, np.float32)
    w_attn = np.asarray(w_attn, np.float32)
    b_attn = np.asarray(b_attn, np.float32)
    w_proj = np.asarray(w_proj, np.float32)
    def to_dr(a):
        # [C, n] -> paired-contraction fp8 layout [p, ko2, s, n], c=ko2*256+s*128+p
        n = a.shape[1]
        q = np.clip(a, -240, 240).astype(F8)
        return np.ascontiguousarray(q.reshape(4, 2, 128, n).transpose(2, 0, 1, 3))

    in_maps = []
    xts = [np.ascontiguousarray(x[b].T).astype(BF16) for b in range(B)]
    xt8s = [to_dr(np.asarray(x[b].T, np.float32)) for b in range(B)]
    for core in range(NCORES):
        b, g = divmod(core, G)
        wqk = np.concatenate(
            [w_attn[:, g * 512:(g + 1) * 512], w_attn[:, 1024 + g * 512:1024 + (g + 1) * 512]],
            axis=1,
        )
        wqk8 = to_dr(wqk * WS)
        wv = w_attn[:, 2048 + g * 512:2048 + (g + 1) * 512].astype(BF16)
        wp = w_proj[g * 512:(g + 1) * 512, :].astype(BF16)
        cols = []
        for cb in range(4):
            cols.append(b_attn[g * 512 + cb * 128: g * 512 + (cb + 1) * 128])
        for cb in range(4):
            cols.append(b_attn[1024 + g * 512 + cb * 128: 1024 + g * 512 + (cb + 1) * 128])
        bqk = (np.stack(cols, axis=1) * WS).astype(np.float32)
        in_maps.append(
            {"xt": xts[b], "xt8": xt8s[b], "wqk8": wqk8, "wv": wv, "wp": wp, "bqk": bqk}
        )
    return in_maps


def kernel(x, w_attn, b_attn, w_proj, b_proj):
    x = np.asarray(x, np.float32)
    w_attn = np.asarray(w_attn, np.float32)
    b_attn = np.asarray(b_attn, np.float32)
    w_proj = np.asarray(w_proj, np.float32)
    b_proj = np.asarray(b_proj, np.float32)

    nc = get_nc()
    in_maps = make_in_maps(x, w_attn, b_attn, w_proj)

    res = run_bass_kernel_spmd(nc, in_maps, list(range(NCORES))).results

    # v-bias contributes b_v @ w_proj to every output row; add with b_proj.
    bias_total = (b_proj + b_attn[2048:] @ w_proj).astype(np.float32)
    out = np.empty((B, T, C), np.float32)
    for b in range(B):
        out[b] = (
            res[G * b]["out"].astype(np.float32)
            + res[G * b + 1]["out"].astype(np.float32)
            + bias_total
        )
    return out
